# revision 1
# baseline (speedup 1.0000x reference)
"""Trainium2 Bass kernel for nn_K_attention_12086037971047.

out[b] = x[b] + Km[b] @ x[b],  Km = exp(-r_sigma * d2(x_b)) with zero diagonal.

Key identity: Km = diag(a) . E . diag(a) with
  a_i = exp(-sigma*||x_i||^2),  E = exp(2*sigma * x x^T)  (E symmetric).
Masked-diagonal output:
  out = coef (.) x + a (.) (E @ (a (.) x)),   coef_i = 1 - a_i^2 * exp(2*sigma*||x_i||^2)
(the coef term subtracts the j==i contribution of the unmasked sum).

Sharding: data-parallel over B: 16 batches -> 8 cores x 2 batches.

Per batch on each core (T=2048, C=64, P=128):
  phase 1: G row-block [128 x 2048] = x_blk x^T via f32r matmuls (K=C=64),
           E row = exp(2 sigma G) on the ACT engine (PSUM -> SBUF)
  phase 2: zT [64 x 2048] += y_blk^T . E_row (f32r, K=128), accumulated in PSUM
           over the 16 row blocks (zT = (E @ y)^T by symmetry of E)
  epilogue: transpose zT back to row layout on the PE, combine with coef/a,
           DMA out contiguously.
"""

import numpy as np

import concourse.bass as bass
import concourse.mybir as mybir
import concourse.tile as tile
from concourse import bacc
from concourse.bass_utils import run_bass_kernel_spmd
from concourse.masks import make_identity

B, T, C = 16, 2048, 64
N_CORES = 8
B_LOC = B // N_CORES  # batches per core
P = 128
NB = T // P  # 16 row blocks
FC = 512  # psum chunk (one 2KB fp32 bank)
NCH = T // FC  # 4 chunks

F32 = mybir.dt.float32
F32R = mybir.dt.float32r
AF = mybir.ActivationFunctionType
OP = mybir.AluOpType


def _emit(tc: tile.TileContext, x, rs, out, reps: int = 1):
    nc = tc.nc
    import contextlib

    with contextlib.ExitStack() as ctx:
        singles = ctx.enter_context(tc.tile_pool(name="singles", bufs=1))
        sb = ctx.enter_context(tc.tile_pool(name="sb", bufs=2))
        ps = ctx.enter_context(tc.tile_pool(name="ps", bufs=1, space="PSUM"))

        # --- constants ---
        sig = singles.tile([P, 1], F32)
        nc.sync.dma_start(sig, rs[:].to_broadcast([P, 1]))
        neg_sig = singles.tile([P, 1], F32)
        nc.scalar.mul(neg_sig, sig, -1.0)
        two_sig = singles.tile([P, 1], F32)
        nc.scalar.mul(two_sig, sig, 2.0)
        ident = singles.tile([P, P], F32)
        make_identity(nc, ident)

        for b in [bb for _ in range(reps) for bb in range(B_LOC)]:
            # --- load x in row layout: partition p holds rows o*128+p ---
            x_rows = sb.tile([P, NB, C], F32, tag="x_rows")
            nc.sync.dma_start(x_rows, x[b].rearrange("(o p) c -> p o c", p=P))

            # --- per-row stats: sq, a=exp(-s*sq), coef = 1 - a^2*exp(2s*sq) ---
            xsq = sb.tile([P, NB, C], F32, tag="xsq")
            nc.vector.tensor_mul(xsq, x_rows, x_rows)
            sq = sb.tile([P, NB], F32, tag="sq")
            nc.vector.tensor_reduce(sq, xsq, axis=mybir.AxisListType.X, op=OP.add)
            a_t = sb.tile([P, NB], F32, tag="a_t")
            nc.scalar.activation(a_t, sq, AF.Exp, scale=neg_sig)
            e_diag = sb.tile([P, NB], F32, tag="e_diag")
            nc.scalar.activation(e_diag, sq, AF.Exp, scale=two_sig)
            coef = sb.tile([P, NB], F32, tag="coef")
            nc.vector.tensor_mul(coef, a_t, a_t)
            nc.vector.tensor_mul(coef, coef, e_diag)
            # coef = 1 - coef
            nc.vector.tensor_scalar(coef, coef, -1.0, 1.0, OP.mult, OP.add)

            # --- y = a (.) x (row-scaled) ---
            y_t = sb.tile([P, NB, C], F32R, tag="y_t")
            nc.vector.tensor_tensor(
                y_t, x_rows, a_t[:, :, None].to_broadcast([P, NB, C]), OP.mult
            )

            # --- xT [C, T] via PE transposes ---
            xT = sb.tile([C, T], F32R, tag="xT")
            for o in range(NB):
                tp = ps.tile([C, P], F32, tag="tp", bufs=2)
                nc.tensor.transpose(tp, x_rows[:, o, :], ident)
                nc.vector.tensor_copy(xT[:, o * P : (o + 1) * P], tp)

            # --- main loop: E row blocks + zT accumulation ---
            zT = ps.tile([C, T], F32, tag="zT", bufs=1)
            for o in range(NB):
                e_sb = sb.tile([P, T], F32R, tag="e_sb")
                for ch in range(NCH):
                    g_ps = ps.tile([P, FC], F32, tag="g", bufs=2)
                    nc.tensor.matmul(
                        g_ps,
                        lhsT=xT[:, o * P : (o + 1) * P],
                        rhs=xT[:, ch * FC : (ch + 1) * FC],
                        start=True,
                        stop=True,
                    )
                    nc.scalar.activation(
                        e_sb[:, ch * FC : (ch + 1) * FC], g_ps, AF.Exp, scale=two_sig
                    )
                for ch in range(NCH):
                    nc.tensor.matmul(
                        zT[:, ch * FC : (ch + 1) * FC],
                        lhsT=y_t[:, o, :],
                        rhs=e_sb[:, ch * FC : (ch + 1) * FC],
                        start=(o == 0),
                        stop=(o == NB - 1),
                        skip_group_check=True,
                    )

            # --- epilogue: zT -> rows, combine, store ---
            zT_sb = sb.tile([C, T], F32, tag="zT_sb")
            nc.vector.tensor_copy(zT_sb, zT)
            z_rows = sb.tile([P, NB, C], F32, tag="z_rows")
            for o in range(NB):
                tp2 = ps.tile([P, C], F32, tag="tp", bufs=2)
                nc.tensor.transpose(tp2, zT_sb[:, o * P : (o + 1) * P], ident[:C, :C])
                nc.vector.tensor_copy(z_rows[:, o, :], tp2)

            out_sb = sb.tile([P, NB, C], F32, tag="out_sb")
            nc.vector.tensor_tensor(
                out_sb, z_rows, a_t[:, :, None].to_broadcast([P, NB, C]), OP.mult
            )
            xc = sb.tile([P, NB, C], F32, tag="xc")
            nc.vector.tensor_tensor(
                xc, x_rows, coef[:, :, None].to_broadcast([P, NB, C]), OP.mult
            )
            nc.vector.tensor_add(out_sb, out_sb, xc)
            nc.sync.dma_start(out[b].rearrange("(o p) c -> p o c", p=P), out_sb)


def build(reps: int = 1):
    nc = bacc.Bacc("TRN2", target_bir_lowering=False)
    x = nc.dram_tensor("x", [B_LOC, T, C], F32, kind="ExternalInput")
    rs = nc.dram_tensor("r_sigma", [1], F32, kind="ExternalInput")
    out = nc.dram_tensor("out", [B_LOC, T, C], F32, kind="ExternalOutput")
    with tile.TileContext(nc) as tc:
        _emit(tc, x, rs, out, reps=reps)
    nc.compile()
    return nc


_NC = None


def _get_nc():
    global _NC
    if _NC is None:
        _NC = build()
    return _NC


def kernel(x: np.ndarray, r_sigma: np.ndarray) -> np.ndarray:
    x = np.ascontiguousarray(x, dtype=np.float32)
    r_sigma = np.ascontiguousarray(r_sigma, dtype=np.float32)
    nc = _get_nc()
    in_maps = [
        {"x": x[i * B_LOC : (i + 1) * B_LOC], "r_sigma": r_sigma}
        for i in range(N_CORES)
    ]
    res = run_bass_kernel_spmd(nc, in_maps, core_ids=list(range(N_CORES)))
    return np.concatenate([r["out"] for r in res.results], axis=0)



# revision 2
# speedup vs baseline: 786.2639x; 786.2639x over previous
"""Trainium2 Bass kernel v6 for nn_K_attention_12086037971047.

out = a (.) (E @ (a (.) x)),  a = exp(-sigma*||x||^2), E = exp(2 sigma x x^T)
(the unmasked diagonal of E reproduces the "+x" term exactly).

Per core (2 batches): row-permuted contiguous DMA; bf16 matmuls; phase 1
computes only upper-triangle G strips (2x row-tiled) into double-buffered
PSUM windows; ACT exps them into a full-row E tile; the lower triangle is
mirrored by the DMA xbar transpose; phase 2 streams full E row-strips
(2x col-tiled by strip parity) into one full-width PSUM accumulator,
trailing ACT by two strips; the final zT -> row-layout transpose also runs
on the DMA xbar.

PSUM budget (16KB/partition): G windows [128,1024]f32 x2 (8KB) +
zps [128,2048]f32 (8KB, shared by tag with the stage-A transpose scratch).
"""

import numpy as np

import concourse.bass as bass
import concourse.mybir as mybir
import concourse.tile as tile
from concourse import bacc
from concourse.bass_utils import run_bass_kernel_spmd
from concourse.masks import make_identity

B, T, C = 16, 2048, 64
N_CORES = 8
B_LOC = B // N_CORES
P = 128
NB = T // P  # 16

F32 = mybir.dt.float32
BF16 = mybir.dt.bfloat16
AF = mybir.ActivationFunctionType
OP = mybir.AluOpType

WIN = 1024
BANK = 512


def _strip_windows(o):
    res = []
    g = P * o
    while g < T:
        n = min(T - g, WIN)
        res.append((g, n))
        g += n
    return res


def _emit(tc: tile.TileContext, x, rs, out, reps: int = 1):
    nc = tc.nc
    import contextlib

    with contextlib.ExitStack() as ctx:
        singles = ctx.enter_context(tc.tile_pool(name="singles", bufs=1))
        sb = ctx.enter_context(tc.tile_pool(name="sb", bufs=2))
        ps = ctx.enter_context(tc.tile_pool(name="ps", bufs=1, space="PSUM"))

        sig = singles.tile([P, 1], F32)
        nc.sync.dma_start(sig, rs[:].to_broadcast([P, 1]))
        neg_sig = singles.tile([P, 1], F32)
        nc.scalar.mul(neg_sig, sig, -1.0)
        two_sig = singles.tile([P, 1], F32)
        nc.scalar.mul(two_sig, sig, 2.0)
        ident = singles.tile([P, P], BF16)
        make_identity(nc, ident)

        for _ in range(reps):
            stage = {}
            # ---------- stage A (both batches): load, stats, casts, xT ----
            for b in range(B_LOC):
                x_rows = sb.tile([P, NB, C], F32, tag="x_rows")
                # row t = 16*p + o  (contiguous 4KB per partition)
                nc.sync.dma_start(x_rows, x[b].rearrange("(p o) c -> p o c", p=P))

                xsq = sb.tile([P, NB, C], F32, tag="xsq")
                nc.vector.tensor_mul(xsq, x_rows, x_rows)
                sq = sb.tile([P, NB], F32, tag="sq")
                nc.vector.tensor_reduce(sq, xsq, axis=mybir.AxisListType.X, op=OP.add)
                a_t = sb.tile([P, NB], F32, tag="a_t")
                nc.scalar.activation(a_t, sq, AF.Exp, scale=neg_sig)

                x_bf = sb.tile([P, NB, C], BF16, tag="x_bf")
                nc.vector.tensor_copy(x_bf, x_rows)
                y_bf = sb.tile([P, NB, C], BF16, tag="y_bf")
                nc.vector.tensor_tensor(
                    y_bf, x_rows, a_t[:, :, None].to_broadcast([P, NB, C]), OP.mult
                )

                # xT on BOTH partition halves (for 2x row-tiled phase 1);
                # the PE-transpose scratch borrows the zps tag's banks
                xTb = sb.tile([P, T], BF16, tag="xTb")
                for g4 in range(NB // 4):
                    tp = ps.tile([P, 512], BF16, tag="zT", bufs=1)
                    for q in range(4):
                        o = 4 * g4 + q
                        qs = slice(q * P, (q + 1) * P)
                        nc.tensor.transpose(tp[0:C, qs], x_bf[:, o, :], ident)
                        nc.tensor.transpose(tp[C:P, qs], x_bf[:, o, :], ident)
                    nc.vector.tensor_copy(xTb[:, g4 * 512 : (g4 + 1) * 512], tp)
                stage[b] = (x_rows, a_t, y_bf, xTb)

            # ---------- stage B (both batches) -----------------------------
            for b in range(B_LOC):
                x_rows, a_t, y_bf, xTb = stage[b]

                E = sb.tile([P, NB, T], BF16, tag="E")
                zps = ps.tile([P, T], F32, tag="zT", bufs=1)

                def emit_ph2(j):
                    opart = slice(0, C) if j % 2 == 0 else slice(C, P)
                    for cpos in range(0, T, BANK):
                        nc.tensor.matmul(
                            zps[opart, cpos : cpos + BANK],
                            lhsT=y_bf[:, j, :],
                            rhs=E[:, j, cpos : cpos + BANK],
                            start=(j < 2),
                            stop=(j >= NB - 2),
                            skip_group_check=True,
                        )

                ci = 0
                for o in range(NB):
                    for (g0, n) in _strip_windows(o):
                        gwt = ps.tile([P, WIN], F32, tag="g", bufs=2)
                        cpos = 0
                        while cpos < n:
                            cn = min(BANK - (cpos % BANK), n - cpos)
                            half = slice(0, C) if ci % 2 == 0 else slice(C, P)
                            ci += 1
                            nc.tensor.matmul(
                                gwt[:, cpos : cpos + cn],
                                lhsT=xTb[half, o * P : (o + 1) * P],
                                rhs=xTb[half, g0 + cpos : g0 + cpos + cn],
                                start=True,
                                stop=True,
                            )
                            cpos += cn
                        nc.scalar.activation(
                            E[:, o, g0 : g0 + n], gwt[:, :n], AF.Exp, scale=two_sig
                        )
                    if o < NB - 1:
                        nc.sync.dma_start(
                            E[:, o + 1 : NB, o * P : o * P + P],
                            E[:, o, (o + 1) * P : T],
                            transpose=True,
                        )
                    if o >= 2:
                        emit_ph2(o - 2)
                emit_ph2(NB - 2)
                emit_ph2(NB - 1)

                # epilogue per column-half: evac zps -> bf16, DMA-xbar
                # transpose both parity planes to row layout, merge, scale
                zT_sb = sb.tile([P, T], BF16, tag="zT_sb")
                zA = sb.tile([P, NB, C], BF16, tag="zA")
                zB = sb.tile([P, NB, C], BF16, tag="zB")
                out_sb = sb.tile([P, NB, C], F32, tag="out_sb")
                for h in range(4):
                    sl = slice(h * 512, (h + 1) * 512)
                    osl = slice(4 * h, 4 * h + 4)
                    nc.vector.tensor_copy(zT_sb[:, sl], zps[:, sl])
                    # zA[s, 4h+k, c] = zT_sb[c, 512h + 128k + s]
                    nc.sync.dma_start(zA[:, osl, :], zT_sb[0:C, sl], transpose=True)
                    nc.sync.dma_start(zB[:, osl, :], zT_sb[C:P, sl], transpose=True)
                    zsum = sb.tile([P, 4, C], F32, tag="zsum")
                    nc.vector.tensor_add(zsum, zA[:, osl, :], zB[:, osl, :])
                    nc.vector.tensor_tensor(
                        out_sb[:, osl, :],
                        zsum,
                        a_t[:, osl, None].to_broadcast([P, 4, C]),
                        OP.mult,
                    )
                    nc.sync.dma_start(
                        out[b].rearrange("(p o) c -> p o c", p=P)[:, osl, :],
                        out_sb[:, osl, :],
                    )


def build(reps: int = 1):
    nc = bacc.Bacc("TRN2", target_bir_lowering=False)
    x = nc.dram_tensor("x", [B_LOC, T, C], F32, kind="ExternalInput")
    rs = nc.dram_tensor("r_sigma", [1], F32, kind="ExternalInput")
    out = nc.dram_tensor("out", [B_LOC, T, C], F32, kind="ExternalOutput")
    with tile.TileContext(nc) as tc:
        _emit(tc, x, rs, out, reps=reps)
    nc.compile()
    return nc


_NC = None


def _get_nc():
    global _NC
    if _NC is None:
        _NC = build()
    return _NC


def kernel(x: np.ndarray, r_sigma: np.ndarray) -> np.ndarray:
    x = np.ascontiguousarray(x, dtype=np.float32)
    r_sigma = np.ascontiguousarray(r_sigma, dtype=np.float32)
    nc = _get_nc()
    in_maps = [
        {"x": x[i * B_LOC : (i + 1) * B_LOC], "r_sigma": r_sigma}
        for i in range(N_CORES)
    ]
    res = run_bass_kernel_spmd(nc, in_maps, core_ids=list(range(N_CORES)))
    return np.concatenate([r["out"] for r in res.results], axis=0)


# revision 7
# speedup vs baseline: 814.7263x; 1.0362x over previous
"""Trainium2 Bass kernel v6 for nn_K_attention_12086037971047.

out = a (.) (E @ (a (.) x)),  a = exp(-sigma*||x||^2), E = exp(2 sigma x x^T)
(the unmasked diagonal of E reproduces the "+x" term exactly).

Per core (2 batches): row-permuted contiguous DMA; bf16 matmuls; phase 1
computes only upper-triangle G strips (2x row-tiled) into double-buffered
PSUM windows; ACT exps them into a full-row E tile; the lower triangle is
mirrored by the DMA xbar transpose; phase 2 streams full E row-strips
(2x col-tiled by strip parity) into one full-width PSUM accumulator,
trailing ACT by two strips; the final zT -> row-layout transpose also runs
on the DMA xbar.

PSUM budget (16KB/partition): G windows [128,1024]f32 x2 (8KB) +
zps [128,2048]f32 (8KB, shared by tag with the stage-A transpose scratch).
"""

import numpy as np

import concourse.bass as bass
import concourse.mybir as mybir
import concourse.tile as tile
from concourse import bacc
from concourse.bass_utils import run_bass_kernel_spmd
from concourse.masks import make_identity

B, T, C = 16, 2048, 64
N_CORES = 8
B_LOC = B // N_CORES
P = 128
NB = T // P  # 16

F32 = mybir.dt.float32
BF16 = mybir.dt.bfloat16
AF = mybir.ActivationFunctionType
OP = mybir.AluOpType

WIN = 1024
BANK = 512


def _strip_windows(o):
    res = []
    g = P * o
    while g < T:
        n = min(T - g, WIN)
        res.append((g, n))
        g += n
    return res


def _emit(tc: tile.TileContext, x, rs, out, reps: int = 1):
    nc = tc.nc
    import contextlib

    with contextlib.ExitStack() as ctx:
        singles = ctx.enter_context(tc.tile_pool(name="singles", bufs=1))
        sb = ctx.enter_context(tc.tile_pool(name="sb", bufs=2))
        ps = ctx.enter_context(tc.tile_pool(name="ps", bufs=1, space="PSUM"))

        sig = singles.tile([P, 1], F32)
        nc.sync.dma_start(sig, rs[:].to_broadcast([P, 1]))
        neg_sig = singles.tile([P, 1], F32)
        nc.scalar.mul(neg_sig, sig, -1.0)
        two_sig = singles.tile([P, 1], F32)
        nc.scalar.mul(two_sig, sig, 2.0)
        ident = singles.tile([P, P], BF16)
        make_identity(nc, ident)
        # preload the exp table set before any real work needs it
        warm = singles.tile([P, 1], F32)
        nc.scalar.activation(warm, sig, AF.Exp)

        for _ in range(reps):
            stage = {}
            # ---------- stage A (both batches): load, stats, casts, xT ----
            for b in range(B_LOC):
                x_rows = sb.tile([P, NB, C], F32, tag="x_rows")
                # row t = 16*p + o  (contiguous 2KB chunks per partition);
                # two DMAs so the bf16 cast can start after the first half
                xv = x[b].rearrange("(p o) c -> p o c", p=P)
                nc.sync.dma_start(x_rows[:, 0 : NB // 2, :], xv[:, 0 : NB // 2, :])
                nc.sync.dma_start(x_rows[:, NB // 2 :, :], xv[:, NB // 2 :, :])

                # cast + transposes first: they gate phase 1 / ACT startup
                x_bf = sb.tile([P, NB, C], BF16, tag="x_bf")
                nc.vector.tensor_copy(x_bf, x_rows)

                # xT on BOTH partition halves (for 2x row-tiled phase 1);
                # the PE-transpose scratch borrows the zps tag's banks
                xTb = sb.tile([P, T], BF16, tag="xTb")
                tpb = ps.tile([P, 4, 512], BF16, tag="zT", bufs=1)
                for g4 in range(NB // 4):
                    for q in range(4):
                        o = 4 * g4 + q
                        qs = slice(q * P, (q + 1) * P)
                        nc.tensor.transpose(tpb[0:C, g4, qs], x_bf[:, o, :], ident)
                        nc.tensor.transpose(tpb[C:P, g4, qs], x_bf[:, o, :], ident)
                    nc.vector.tensor_copy(
                        xTb[:, g4 * 512 : (g4 + 1) * 512], tpb[:, g4]
                    )

                xsq = sb.tile([P, NB, C], F32, tag="xsq")
                nc.vector.tensor_mul(xsq, x_rows, x_rows)
                sq = sb.tile([P, NB], F32, tag="sq")
                nc.vector.tensor_reduce(sq, xsq, axis=mybir.AxisListType.X, op=OP.add)
                a_t = sb.tile([P, NB], F32, tag="a_t")
                nc.scalar.activation(a_t, sq, AF.Exp, scale=neg_sig)
                y_bf = sb.tile([P, NB, C], BF16, tag="y_bf")
                nc.vector.tensor_tensor(
                    y_bf, x_rows, a_t[:, :, None].to_broadcast([P, NB, C]), OP.mult
                )
                stage[b] = (x_rows, a_t, y_bf, xTb)

            # ---------- stage B (both batches) -----------------------------
            for b in range(B_LOC):
                x_rows, a_t, y_bf, xTb = stage[b]

                E = sb.tile([P, NB, T], BF16, tag="E")
                zps = ps.tile([P, T], F32, tag="zT", bufs=1)

                def emit_ph2(j):
                    opart = slice(0, C) if j % 2 == 0 else slice(C, P)
                    for cpos in range(0, T, BANK):
                        nc.tensor.matmul(
                            zps[opart, cpos : cpos + BANK],
                            lhsT=y_bf[:, j, :],
                            rhs=E[:, j, cpos : cpos + BANK],
                            start=(j < 2),
                            stop=(j >= NB - 2),
                            skip_group_check=True,
                        )

                ci = 0
                for o in range(NB):
                    for (g0, n) in _strip_windows(o):
                        gwt = ps.tile([P, WIN], F32, tag="g", bufs=2)
                        cpos = 0
                        while cpos < n:
                            cn = min(BANK - (cpos % BANK), n - cpos)
                            half = slice(0, C) if ci % 2 == 0 else slice(C, P)
                            ci += 1
                            nc.tensor.matmul(
                                gwt[:, cpos : cpos + cn],
                                lhsT=xTb[half, o * P : (o + 1) * P],
                                rhs=xTb[half, g0 + cpos : g0 + cpos + cn],
                                start=True,
                                stop=True,
                            )
                            cpos += cn
                        nc.scalar.activation(
                            E[:, o, g0 : g0 + n], gwt[:, :n], AF.Exp, scale=two_sig
                        )
                    if o < NB - 1:
                        nc.sync.dma_start(
                            E[:, o + 1 : NB, o * P : o * P + P],
                            E[:, o, (o + 1) * P : T],
                            transpose=True,
                        )
                    if o >= 2:
                        emit_ph2(o - 2)
                emit_ph2(NB - 2)
                emit_ph2(NB - 1)

                # epilogue per column-half: evac zps -> bf16, DMA-xbar
                # transpose both parity planes to row layout, merge, scale
                zT_sb = sb.tile([P, T], BF16, tag="zT_sb")
                zA = sb.tile([P, NB, C], BF16, tag="zA")
                zB = sb.tile([P, NB, C], BF16, tag="zB")
                out_sb = sb.tile([P, NB, C], F32, tag="out_sb")
                for h in range(4):
                    sl = slice(h * 512, (h + 1) * 512)
                    osl = slice(4 * h, 4 * h + 4)
                    nc.vector.tensor_copy(zT_sb[:, sl], zps[:, sl])
                    # zA[s, 4h+k, c] = zT_sb[c, 512h + 128k + s]
                    nc.sync.dma_start(zA[:, osl, :], zT_sb[0:C, sl], transpose=True)
                    nc.sync.dma_start(zB[:, osl, :], zT_sb[C:P, sl], transpose=True)
                    zsum = sb.tile([P, 4, C], F32, tag="zsum")
                    nc.vector.tensor_add(zsum, zA[:, osl, :], zB[:, osl, :])
                    nc.vector.tensor_tensor(
                        out_sb[:, osl, :],
                        zsum,
                        a_t[:, osl, None].to_broadcast([P, 4, C]),
                        OP.mult,
                    )
                    nc.sync.dma_start(
                        out[b].rearrange("(p o) c -> p o c", p=P)[:, osl, :],
                        out_sb[:, osl, :],
                    )


def build(reps: int = 1):
    nc = bacc.Bacc("TRN2", target_bir_lowering=False)
    x = nc.dram_tensor("x", [B_LOC, T, C], F32, kind="ExternalInput")
    rs = nc.dram_tensor("r_sigma", [1], F32, kind="ExternalInput")
    out = nc.dram_tensor("out", [B_LOC, T, C], F32, kind="ExternalOutput")
    with tile.TileContext(nc) as tc:
        _emit(tc, x, rs, out, reps=reps)
    nc.compile()
    return nc


_NC = None


def _get_nc():
    global _NC
    if _NC is None:
        _NC = build()
    return _NC


def kernel(x: np.ndarray, r_sigma: np.ndarray) -> np.ndarray:
    x = np.ascontiguousarray(x, dtype=np.float32)
    r_sigma = np.ascontiguousarray(r_sigma, dtype=np.float32)
    nc = _get_nc()
    in_maps = [
        {"x": x[i * B_LOC : (i + 1) * B_LOC], "r_sigma": r_sigma}
        for i in range(N_CORES)
    ]
    res = run_bass_kernel_spmd(nc, in_maps, core_ids=list(range(N_CORES)))
    return np.concatenate([r["out"] for r in res.results], axis=0)
